# revision 23
# baseline (speedup 1.0000x reference)
# BinaryLinear on 8 Trainium2 NeuronCores.
#
# y = x @ sign(W)^T + bias for x [8192, 4096] f32, W [4096, 4096] f32.
#
# Sharding: data-parallel over the 8192 tokens (1024 per core). Each core
# runs one [K=4096, M=1024] x [K=4096, N=4096] matmul entirely in fp8e4m3
# DoubleRow perf mode (the PE virtualizes to 128x256: 0.5 cycles/moving-row,
# 2x the bf16 FLOP rate, 4x per instruction), with a two-term x quantization:
#   - batch A: hi = e4m3(x), full K=4096, against full W.
#   - batch B: lo = e4m3(x - hi) on the first KLO = N_LO*256 columns,
#     against the same W rows (error-feedback correction).
# +-1 weights are exact in fp8; accumulation is fp32 in PSUM across both
# batches. Corrected columns carry only the second-order residual, so
# rel err ~= sqrt(1 - KLO/4096)*0.0266; N_LO=8 measures 0.0187 on the
# reference data vs the 2e-2 gate.
#
# The device loop is fully hand-rolled (no composable kernel). Per-core
# work = 8 W out-chunks x 2 token halves x 4 token subtiles, each a
# [128 tok, 512 out] PSUM tile accumulated over K=4096+2048 in 24
# DoubleRow matmuls. Schedule (all timings vs the TimelineSim cost model):
#  - opening block = (W0+W1) x token-half-0 across all 8 PSUM banks,
#    with hi and lo k-tiles interleaved 4:2 to mirror the staging stream:
#    hi k-tiles cost 1.09us of DMA per 0.85us of PE while lo k-tiles
#    (which reuse already-staged W) cost 0.36us per 0.85us, so the mix
#    consumes at exactly the 360GB/s fully-serialized DMA rate and the PE
#    never starves after its first matmul. Staging DMAs are emitted in
#    exact consumption order as (x0,W0,W1) k-chunk triplets of ~256KB
#    (smaller chunks go HWDGE-bound at 625ns/DMA and lose bandwidth),
#    with one xl chunk after every two triplets. The last two hi k-tiles
#    run chain-major so the 8 bank evictions stagger and the next block
#    starts stall-free.
#  - remaining blocks (W0,t1),(W1,t1),(W2..W6)x(t0,t1) run k-outer on 4
#    banks each, parity-alternating; the PE stream is gapless here. Later
#    W-chunk staging is emitted interleaved into the matmul stream so
#    block stores don't head-of-line block behind queued staging on the
#    sync queue. Evictions alternate DVE/Act; one PSUM pool with 8
#    explicitly-tagged banks is used across all phases (a fresh pool at
#    the epilogue would cost a ~1.4us WAR seam on pool churn).
#  - the last W chunk runs k-inner per subtile chain so each eviction+
#    store overlaps the next chain's matmuls; the final subtile is sliced
#    into 256/192/64-col strips on separate banks (PSUM WAR tracking is
#    tile-granular), and the last two strips evict into one f16 tile and
#    leave as a single store, so the serial tail after the last matmul
#    pays one small eviction + one DMA-issue latency only.
#  - the cost model's PE p-state ramp tracker resets when the PE
#    starves, and the clock runs at half rate for the first 3us of each
#    continuous run. A 6-matmul dependency-free warm-up chain keeps the
#    PE busy from t~=1.5us until the first staged chunks arrive, so the
#    real stream runs at full clock from its ~2nd matmul (measured:
#    warm-up recovers ~2.6us vs starting the ramp cold).
#
# Execution goes through bass2jax/PJRT (axon): one jitted shard_map over the
# 8-core mesh. The donated output backing buffer is created on-device so no
# zero-filled bytes cross the host->device link.

import numpy as np
import ml_dtypes

N_TOKENS = 8192
IN_F = 4096
OUT_F = 4096
N_CORES = 8
TOK_SHARD = N_TOKENS // N_CORES

N_LO = 8  # number of 256-wide K chunks getting the lo correction term
KLO = N_LO * 256

_C = {}


OUT_DT = "float16"  # device-side output dtype (upcast to f32 on host).
# f16 keeps D2H small; rounding f32 PSUM results to f16 adds ~3e-4 relative
# error rms on top of the x-quantization error — negligible.


def _build_nc(
    out_dt=None,
    n_lo=None,
    repeats=1,
    n_warm=6,
    n_warm_fine=0,  # extra 128-wide warm matmuls to tune the junction
    ev_tags=16,
    fine_sub=4,  # staging chunk granularity (k-subtiles) for the startup phase
    tail_splits=(256, 192, 64),
):
    import contextlib

    import concourse.bass as bass
    import concourse.mybir as mybir
    import concourse.tile as tile
    from concourse import bacc

    out_dt = out_dt or OUT_DT
    n_lo = N_LO if n_lo is None else n_lo
    klo = n_lo * 256
    KO, KOL = IN_F // 128, klo // 128  # 32, 16 k-subtiles
    KT, KTL = KO // 2, KOL // 2  # 16 hi + 8 lo k-tiles (256 K each)
    MCH, NCH = TOK_SHARD // 512, OUT_F // 512  # 2 token halves, 8 W chunks
    DR = mybir.MatmulPerfMode.DoubleRow
    f8 = mybir.dt.float8e4
    odt = getattr(mybir.dt, out_dt)

    nc = bacc.Bacc("TRN2", target_bir_lowering=False, debug=False)
    # All inputs arrive partition-major and consumption-chunk-major
    # (x by m-half, W by n-512-chunk); per-partition rows are contiguous
    # within a chunk so every staging DMA moves large descriptors.
    x8_t = nc.dram_tensor(
        "x8_t", [128, MCH * KO * 512], f8, kind="ExternalInput"
    ).ap()
    xl_t = (
        nc.dram_tensor(
            "xl_t", [128, MCH * KOL * 512], f8, kind="ExternalInput"
        ).ap()
        if klo
        else None
    )
    w_t = nc.dram_tensor(
        "w_t", [128, NCH * KO * 512], f8, kind="ExternalInput"
    ).ap()
    y = nc.dram_tensor(
        "y", [TOK_SHARD, OUT_F], odt, kind="ExternalOutput"
    ).ap()

    with tile.TileContext(nc) as tc:
        with contextlib.ExitStack() as es:
            io_pool = es.enter_context(tc.tile_pool(name="io_pool", bufs=1))
            ps_pool = es.enter_context(
                tc.tile_pool(name="ps", bufs=1, space="PSUM")
            )
            ev_pool = es.enter_context(tc.tile_pool(name="ev", bufs=1))

            w4 = w_t.rearrange("p (nc ko n) -> p nc ko n", nc=NCH, ko=KO)
            x84 = x8_t.rearrange("p (mc ko m) -> p mc ko m", mc=MCH, ko=KO)
            xl4 = (
                xl_t.rearrange("p (mc ko m) -> p mc ko m", mc=MCH, ko=KOL)
                if klo
                else None
            )
            y3 = y.rearrange("(po pi) f -> pi po f", pi=128)

            if n_warm:
                # Dependency-free dummy matmuls; they run while the first
                # staging DMAs are in flight and pin pe_busy_start to ~0.6us
                # so the p-state ramp completes under the DMA-bound phase.
                wt_ = io_pool.tile(
                    [128, 512], mybir.dt.bfloat16, name="warm_in", tag="warm_in"
                )
                nc.vector.memset(wt_[:], 1.0)
                wp = ps_pool.tile(
                    [128, 512], mybir.dt.float32, name="warm_ps", tag="bank0"
                )
                for _ in range(n_warm):
                    nc.tensor.matmul(
                        wp[:], wt_[:, :128], wt_[:], start=True, stop=True
                    )
                for _ in range(n_warm_fine):
                    nc.tensor.matmul(
                        wp[:, :128], wt_[:, :128], wt_[:, :128],
                        start=True, stop=True,
                    )

            for rep in range(repeats):
                w_sbufs = [
                    io_pool.tile(
                        [128, KO, 512], f8, name=f"w_sbuf{c}", tag=f"w_sbuf{c}"
                    )
                    for c in range(NCH)
                ]
                x8_sbufs = [
                    io_pool.tile(
                        [128, KO, 512], f8, name=f"x8_sbuf{c}", tag=f"x8_sbuf{c}"
                    )
                    for c in range(MCH)
                ]
                xl_sbufs = [
                    (
                        io_pool.tile(
                            [128, KOL, 512], f8,
                            name=f"xl_sbuf{c}", tag=f"xl_sbuf{c}",
                        )
                        if klo
                        else None
                    )
                    for c in range(MCH)
                ]

                def dma(sb, dr, mi, k0, k1):
                    nc.sync.dma_start(sb[:, k0:k1, :], dr[:, mi, k0:k1, :])

                def dma_fine(sb, dr, mi, k0, k1, step):
                    for k in range(k0, k1, step):
                        dma(sb, dr, mi, k, min(k + step, k1))

                # --- staging, exact consumption order ---------------------
                # The opening block is (W0+W1, tok half 0) across all 8 PSUM
                # banks: its 7MB of staging feeds 20.5us of PE work, nearly
                # matching the 360GB/s serial DMA rate, so the startup
                # deficit is about half of a single-W-chunk opening. Stage
                # (x0, W0, W1) k-chunk triplets, then xl0, then the data for
                # the following (W0,t1)/(W1,t1) blocks, then W2 coarse.
                FS = fine_sub

                def trip(p):
                    # one (x0, W0, W1) k-chunk triplet: 2 hi k-tiles of data
                    dma(x8_sbufs[0], x84, 0, FS * p, FS * (p + 1))
                    dma(w_sbufs[0], w4, 0, FS * p, FS * (p + 1))
                    dma(w_sbufs[1], w4, 1, FS * p, FS * (p + 1))

                # hi triplets (1.09us/chunk for 0.85us of PE per 2 k-tiles)
                # interleaved with cheap xl chunks (0.73us for 1.71us of PE)
                # at 4:2 so consumption matches the serial DMA rate with no
                # interior deficit; the last xl chunk lands before the last
                # triplet so only T7 gates the block's chain-major tail
                for g in range(3):
                    trip(2 * g)
                    trip(2 * g + 1)
                    if klo:
                        dma(xl_sbufs[0], xl4, 0, FS * g, FS * (g + 1))
                trip(6)
                if klo:
                    dma(xl_sbufs[0], xl4, 0, 3 * FS, KOL)
                trip(7)
                dma_fine(x8_sbufs[1], x84, 1, 0, KO, FS)
                if klo:
                    dma_fine(xl_sbufs[1], xl4, 1, 0, KOL, FS)
                dma_fine(w_sbufs[2], w4, 2, 0, KO, KO // 2)

                # eviction temp tiles: f16 [128,512], ev_tags round-robin
                ev_n = [0]

                def evict_store(pt, cols, po, f0, eng_idx):
                    ev = ev_pool.tile(
                        [128, 512], odt,
                        name=f"ev{rep}_{ev_n[0]}",
                        tag=f"ev{ev_n[0] % ev_tags}",
                    )
                    ev_n[0] += 1
                    if eng_idx % 2 == 0:
                        nc.vector.tensor_copy(out=ev[:, :cols], in_=pt[:, :cols])
                    else:
                        nc.scalar.copy(out=ev[:, :cols], in_=pt[:, :cols])
                    nc.sync.dma_start(y3[:, po, bass.ds(f0, cols)], ev[:, :cols])

                # --- opening block: (W0+W1) x tok half 0, all 8 banks ------
                # k-outer over both W chunks so the staging stream paces it;
                # the last two k-tiles run chain-major (each chain finishes
                # its k22/k23 consecutively and evicts immediately), so the
                # W0-side banks 0-3 are free before (W0,t1) needs them.
                KTT = KT + KTL
                mpts = [
                    ps_pool.tile(
                        [128, 512], mybir.dt.float32,
                        name=f"psM{rep}_{wc}_{ms}", tag=f"bank{wc * 4 + ms}",
                    )
                    for wc in range(2)
                    for ms in range(4)
                ]

                def xsrc(h, kt):
                    if kt < KT:
                        return x8_sbufs[h], 2 * kt
                    return xl_sbufs[h], 2 * (kt - KT)

                # k-tile order mirrors the staging interleave (PSUM
                # accumulation is order-independent); hi 14/15 go last,
                # chain-major, so chain stops stagger for the bank handoff
                k_order = []
                for g in range(3):
                    k_order += [4 * g, 4 * g + 1, 4 * g + 2, 4 * g + 3,
                                KT + 2 * g, KT + 2 * g + 1]
                k_order += [12, 13, KT + 6, KT + 7]
                for kt in k_order:
                    src, ks = xsrc(0, kt)
                    for wc in range(2):
                        for ms in range(4):
                            nc.tensor.matmul(
                                mpts[wc * 4 + ms][:],
                                src[:, ks : ks + 2, ms * 128 : (ms + 1) * 128],
                                w_sbufs[wc][:, ks : ks + 2, :],
                                start=(kt == 0),
                                stop=False,
                                perf_mode=DR,
                            )
                for wc in range(2):
                    for ms in range(4):
                        for kt in (KT - 2, KT - 1):
                            src, ks = xsrc(0, kt)
                            nc.tensor.matmul(
                                mpts[wc * 4 + ms][:],
                                src[:, ks : ks + 2, ms * 128 : (ms + 1) * 128],
                                w_sbufs[wc][:, ks : ks + 2, :],
                                start=False,
                                stop=(kt == KT - 1),
                                perf_mode=DR,
                            )
                        evict_store(
                            mpts[wc * 4 + ms], 512, ms, wc * 512, wc * 4 + ms
                        )

                # --- main loop: remaining (chunk, half) blocks, k-outer ----
                # order: (W0,t1), (W1,t1), then (W2..W6) x (t0,t1); 4 banks
                # per block, parity alternating
                main_blocks = [(0, 1), (1, 1)] + [
                    (c, h) for c in range(2, NCH - 1) for h in range(MCH)
                ]
                w_stage = iter(range(3, NCH))  # W3..W7 staged 2 blocks ahead
                for blk, (c, h) in enumerate(main_blocks):
                    if blk % 2 == 0:
                        cs = next(w_stage, None)
                        if cs is not None:
                            # stage a later W chunk (2 coarse DMAs); emitted
                            # here so block stores interleave with staging on
                            # the sync queue instead of queueing behind it
                            dma_fine(w_sbufs[cs], w4, cs, 0, KO, KO // 2)
                    par = blk % 2
                    pts = [
                        ps_pool.tile(
                            [128, 512], mybir.dt.float32,
                            name=f"ps{rep}_{blk}_{ms}",
                            tag=f"bank{par * 4 + ms}",
                        )
                        for ms in range(4)
                    ]
                    for kt in range(KTT):
                        src, ks = xsrc(h, kt)
                        for ms in range(4):
                            nc.tensor.matmul(
                                pts[ms][:],
                                src[:, ks : ks + 2, ms * 128 : (ms + 1) * 128],
                                w_sbufs[c][:, ks : ks + 2, :],
                                start=(kt == 0),
                                stop=(kt == KTT - 1),
                                perf_mode=DR,
                            )
                    for ms in range(4):
                        evict_store(pts[ms], 512, h * 4 + ms, c * 512, ms)

                # --- epilogue: last W chunk, k-inner per subtile chain -----
                # each chain's eviction+store overlaps the next chain's
                # matmuls; the very last token subtile is sliced so the
                # final serial tail is a narrow strip.
                wlast = w_sbufs[NCH - 1]
                chains = [(h, ms) for h in range(MCH) for ms in range(4)]
                last_h, last_ms = chains[-1]
                f0_base = (NCH - 1) * 512

                def chain_mm(pt, h, ms, c0, c1):
                    sl = pt[:, c0:c1]
                    for r in range(KT):
                        nc.tensor.matmul(
                            sl,
                            x8_sbufs[h][:, 2 * r : 2 * r + 2,
                                        ms * 128 : (ms + 1) * 128],
                            wlast[:, 2 * r : 2 * r + 2, c0:c1],
                            start=(r == 0),
                            stop=(klo == 0 and r == KT - 1),
                            perf_mode=DR,
                        )
                    for r in range(KTL):
                        nc.tensor.matmul(
                            sl,
                            xl_sbufs[h][:, 2 * r : 2 * r + 2,
                                        ms * 128 : (ms + 1) * 128],
                            wlast[:, 2 * r : 2 * r + 2, c0:c1],
                            start=False,
                            stop=(r == KTL - 1),
                            perf_mode=DR,
                        )
                    return sl

                def chain(pt, h, ms, c0, c1, eng_idx):
                    sl = chain_mm(pt, h, ms, c0, c1)
                    ev = ev_pool.tile(
                        [128, 512], odt,
                        name=f"ev{rep}_{ev_n[0]}",
                        tag=f"ev{ev_n[0] % ev_tags}",
                    )
                    ev_n[0] += 1
                    # split stores across two issue queues (Act-evicted
                    # tiles store from Act, DVE-evicted from SP) so chain
                    # stores don't head-of-line block on one SEQ at the
                    # kernel tail
                    if eng_idx % 2 == 0:
                        nc.vector.tensor_copy(out=ev[:, : c1 - c0], in_=sl)
                        eng = nc.sync
                    else:
                        nc.scalar.copy(out=ev[:, : c1 - c0], in_=sl)
                        eng = nc.scalar
                    eng.dma_start(
                        y3[:, h * 4 + ms, bass.ds(f0_base + c0, c1 - c0)],
                        ev[:, : c1 - c0],
                    )

                for j, (h, ms) in enumerate(ch for ch in chains
                                            if ch != (last_h, last_ms)):
                    pt = ps_pool.tile(
                        [128, 512], mybir.dt.float32,
                        name=f"ep{rep}_{j}", tag=f"bank{j % 8}",
                    )
                    chain(pt, h, ms, 0, 512, j)
                # final subtile: independent column strips, one PSUM bank
                # each (tile-granular WAR tracking would otherwise serialize
                # each strip's matmuls behind the previous strip's eviction).
                # Strip A stores on its own (Act queue); strips B and C
                # evict on DVE into adjacent slices of ONE f16 tile and go
                # out as a single store, so the critical path after the last
                # matmul pays one small eviction + one uncontended
                # HWDGE+DGE issue (a separate store for C would queue
                # behind a B-side store's HWDGE generation).
                wA, wB, wC = tail_splits
                ptA = ps_pool.tile(
                    [128, 512], mybir.dt.float32,
                    name=f"ep{rep}_lastA", tag="bank7",
                )
                chain(ptA, last_h, last_ms, 0, wA, 1)
                ptB = ps_pool.tile(
                    [128, 512], mybir.dt.float32,
                    name=f"ep{rep}_lastB", tag="bank0",
                )
                slB = chain_mm(ptB, last_h, last_ms, wA, wA + wB)
                ptC = ps_pool.tile(
                    [128, 512], mybir.dt.float32,
                    name=f"ep{rep}_lastC", tag="bank1",
                )
                slC = chain_mm(ptC, last_h, last_ms, wA + wB, 512)
                evBC = ev_pool.tile(
                    [128, 512], odt,
                    name=f"ev{rep}_tail", tag=f"ev{ev_n[0] % ev_tags}",
                )
                ev_n[0] += 1
                nc.vector.tensor_copy(out=evBC[:, :wB], in_=slB)
                nc.vector.tensor_copy(out=evBC[:, wB : wB + wC], in_=slC)
                nc.sync.dma_start(
                    y3[:, last_h * 4 + last_ms, bass.ds(f0_base + wA, wB + wC)],
                    evBC[:, : wB + wC],
                )

    nc.compile()
    return nc


def _get_nc():
    if "nc" not in _C:
        _C["nc"] = _build_nc()
    return _C["nc"]


def _in_names(nc):
    import concourse.mybir as mybir

    partition_name = nc.partition_id_tensor.name if nc.partition_id_tensor else None
    names = []
    for alloc in nc.m.functions[0].allocations:
        if not isinstance(alloc, mybir.MemoryLocationSet):
            continue
        name = alloc.memorylocations[0].name
        if alloc.kind == "ExternalInput" and name != partition_name:
            names.append(name)
    return names


def _get_runner():
    """Compile the 8-core jitted executable once; returns (fn, zeros_fn)."""
    if "runner" in _C:
        return _C["runner"]
    import jax
    import jax.numpy as jnp
    from jax.sharding import Mesh, NamedSharding, PartitionSpec

    import inspect

    try:
        from jax.experimental.shard_map import shard_map
    except ImportError:
        from jax import shard_map
    _rep_kw = (
        {"check_rep": False}
        if "check_rep" in inspect.signature(shard_map).parameters
        else {"check_vma": False}
    )
    import concourse.mybir as mybir
    from concourse import bass2jax
    from concourse.bass2jax import _bass_exec_p, install_neuronx_cc_hook

    nc = _get_nc()
    install_neuronx_cc_hook()

    partition_name = nc.partition_id_tensor.name if nc.partition_id_tensor else None
    in_names, out_names, out_avals = [], [], []
    for alloc in nc.m.functions[0].allocations:
        if not isinstance(alloc, mybir.MemoryLocationSet):
            continue
        name = alloc.memorylocations[0].name
        if alloc.kind == "ExternalInput":
            if name != partition_name:
                in_names.append(name)
        elif alloc.kind == "ExternalOutput":
            out_names.append(name)
            out_avals.append(
                jax.core.ShapedArray(
                    tuple(alloc.tensor_shape), mybir.dt.np(alloc.dtype)
                )
            )
    expect = ["x8_t"] + (["xl_t"] if KLO else []) + ["w_t"]
    assert in_names == expect and out_names == ["y"], (in_names, out_names)
    all_in_names = list(in_names) + list(out_names)
    if partition_name is not None:
        all_in_names.append(partition_name)

    def _body(*args):
        operands = list(args)
        if partition_name is not None:
            operands.append(bass2jax.partition_id_tensor())
        outs = _bass_exec_p.bind(
            *operands,
            out_avals=tuple(out_avals),
            in_names=tuple(all_in_names),
            out_names=tuple(out_names),
            lowering_input_output_aliases=(),
            sim_require_finite=True,
            sim_require_nnan=True,
            nc=nc,
        )
        return tuple(outs)

    devices = jax.devices()[:N_CORES]
    mesh = Mesh(np.asarray(devices), ("core",))
    sharding = NamedSharding(mesh, PartitionSpec("core"))
    n_args = len(in_names) + 1  # inputs + y backing
    in_specs = (PartitionSpec("core"),) * n_args
    out_specs = (PartitionSpec("core"),)
    fn = jax.jit(
        shard_map(_body, mesh=mesh, in_specs=in_specs, out_specs=out_specs,
                  **_rep_kw),
        donate_argnums=(n_args - 1,),
        keep_unused=True,
    )
    out_np_dt = out_avals[0].dtype
    zeros_fn = jax.jit(
        lambda: jnp.zeros((N_TOKENS, OUT_F), out_np_dt),
        out_shardings=sharding,
    )
    _C["runner"] = (fn, zeros_fn, sharding, jax)
    return _C["runner"]


def _pm(a):
    """[K, M] -> [128, (M//512)*(K//128)*512]: partition-major with the
    free dim chunked by 512 outermost (the kernel's staging-DMA order)."""
    K, M = a.shape
    t = a.reshape(K // 128, 128, M // 512, 512).transpose(1, 2, 0, 3)
    return np.ascontiguousarray(t.reshape(128, -1))


def _shard_cols_pm(xt):
    """[K, 8192] -> [8*128, ...] stacked per-core chunk-major shards."""
    return np.concatenate(
        [_pm(xt[:, c * TOK_SHARD : (c + 1) * TOK_SHARD]) for c in range(N_CORES)],
        axis=0,
    )


def _host_prep(x, weight):
    """sign/transpose/cast/shard on the host (cheap vs the matmul).

    Returns the global (8-core stacked) arrays in kernel input order:
    x8_t (fp8 K-slice), xl_t (fp8 lo K-slice), w_t.
    """
    xt = np.ascontiguousarray(np.asarray(x).T)
    hi = xt.astype(ml_dtypes.float8_e4m3)
    parts = [_shard_cols_pm(hi)]
    if KLO:
        lo = (xt[:KLO] - hi[:KLO].astype(np.float32)).astype(
            ml_dtypes.float8_e4m3
        )
        parts.append(_shard_cols_pm(lo))
    wt = _pm(
        np.sign(np.asarray(weight)).T.astype(ml_dtypes.float8_e4m3)
    )
    parts.append(np.concatenate([wt] * N_CORES, axis=0))
    return parts


def _run_spmd_fallback(x, weight):
    """Conservative path through bass_utils.run_bass_kernel_spmd (same
    underlying bass2jax/PJRT execution; pays extra host->device bytes for the
    zero-filled output backing buffers)."""
    from concourse.bass_utils import run_bass_kernel_spmd

    nc = _get_nc()
    xt = np.ascontiguousarray(np.asarray(x).T)
    hi = xt.astype(ml_dtypes.float8_e4m3)
    lo = (
        (xt[:KLO] - hi[:KLO].astype(np.float32)).astype(ml_dtypes.float8_e4m3)
        if KLO
        else None
    )
    wt = _pm(np.sign(np.asarray(weight)).T.astype(ml_dtypes.float8_e4m3))
    in_maps = []
    for c in range(N_CORES):
        sl = slice(c * TOK_SHARD, (c + 1) * TOK_SHARD)
        m = {"x8_t": _pm(hi[:, sl])}
        if lo is not None:
            m["xl_t"] = _pm(lo[:, sl])
        m["w_t"] = wt
        in_maps.append(m)
    res = run_bass_kernel_spmd(nc, in_maps, core_ids=list(range(N_CORES)))
    return np.concatenate([r["y"] for r in res.results], axis=0)


def kernel(x, weight, bias):
    try:
        fn, zeros_fn, sharding, jax = _get_runner()
        parts = _host_prep(x, weight)
        args = [jax.device_put(p, sharding) for p in parts]
        args.append(zeros_fn())
        (yd,) = fn(*args)
        # global [8192, 4096], token order preserved
        y = np.asarray(yd)
    except Exception:
        y = _run_spmd_fallback(x, weight)
    # upcast + bias on host
    y = y.astype(np.float32)
    y += np.asarray(bias, dtype=np.float32)[None, :]
    return y
